# revision 1
# baseline (speedup 1.0000x reference)
import jax
import jax.numpy as jnp
import numpy as np
from functools import partial

# nn_GCN_17008070492360: N=100000, K=16, F=128, H1=64, H2=32, C=10
# Data-parallel over nodes across 8 NeuronCores; W1/W2/Wc replicated.
N, K, F = 100000, 16, 128
EPS = 1e-12
NCORES = 8


def _l2norm(a):
    n = jnp.linalg.norm(a, axis=-1, keepdims=True)
    return a / jnp.maximum(n, EPS)


@partial(jax.pmap, axis_name="i", in_axes=(0, 0, None, None, None))
def _shard_fn(x, neighbor, W1, W2, Wc):
    x1 = _l2norm(x) @ W1.T                      # [n, H1]
    nb1 = _l2norm(neighbor) @ W1.T              # [n, K, H1]
    agg = jax.nn.relu(nb1.sum(axis=1))          # self path (unused downstream)
    nbs = jax.nn.relu(x1[:, None, :] + nb1)     # [n, K, H1]
    x2 = nbs.sum(axis=1) @ W2.T                 # sum_k (relu(...) @ W2.T)
    del agg
    return jax.nn.relu(x2) @ Wc.T               # [n, C]


def kernel(x, neighbor, W1, W2, Wc):
    x = np.asarray(x, dtype=np.float32)
    neighbor = np.asarray(neighbor, dtype=np.float32)
    n_per = N // NCORES
    xs = x.reshape(NCORES, n_per, F)
    nbs = neighbor.reshape(NCORES, n_per, K, F)
    out = _shard_fn(xs, nbs, jnp.asarray(W1), jnp.asarray(W2), jnp.asarray(Wc))
    return np.asarray(out).reshape(N, -1).astype(np.float32)

